# revision 25
# baseline (speedup 1.0000x reference)
"""Trainium2 Bass kernel for nn_AttentionBlock (GroupNorm + single-head spatial
self-attention + residual) on 8 NeuronCores.

Sharding: data-parallel over batch (2) x sequence-parallel over the query
dimension (4 chunks of 1024 of the 4096 spatial tokens). Each core gets the
full image of its batch element, ROTATED so its query chunk sits at token 0
(GroupNorm stats, key/value sets and softmax sums are permutation-invariant
over tokens, so rotation lets all 8 cores run the identical SPMD program).

v2: every large matmul runs in fp8e4m3 with perf_mode=DoubleRow (two K=128
contraction tiles fused per instruction, ~1.44x PE throughput at FD>=512).
All pairs are [128, 2, N] tiles pairing adjacent 128-blocks of the
contraction dim. Measured end-to-end rel err ~9e-3 vs the 2e-2 gate.

  x ships as fp8 channel-pairs (2MB) and is the matmul operand for q/k/v;
  a second bf16 copy of x arrives later (DMA idle mid-kernel) only for the
  residual. GroupNorm stats come from the fp8 x via DVE free-dim reduces
  (sums) + ScalarE Square activations with accum_out (sumsq) streamed
  behind the DMAs; group-combine via tiny PE indicator matmuls as before.
  The GroupNorm scale a[c] folds into fp8 copies of wq/wk/wv (bf16 raw
  weights ship for the tiny bias contracts); the shift b[c] becomes
  per-output-channel constants (rows of w.b) as in v1.

  scores: sT[j,i] = sum_c k[c,j] q[c,i] as 2 DoubleRow matmuls (c-pairs);
  exp on ScalarE with a constant logit shift of -2.5 (softmax is shift
  invariant; the shift keeps exp under fp8 max 240; logit max is ~7.2)
  writing fp8 straight into j-pair tiles. AV + rowsum consume the pairs
  with DoubleRow (vT j-pairs / fp8 ones). Softmax normalization is
  deferred past the wp projection: po -> fp8 'has' pairs scaled by 1/16
  (keeps po under fp8 range; the x16 folds into the 1/r row broadcast,
  whose ones-row carries value 16). Final y = py*(16/r) + (x + yb).
"""

import sys
from contextlib import ExitStack

if "/opt/trn_rl_repo" not in sys.path:
    sys.path.insert(0, "/opt/trn_rl_repo")

import numpy as np

import concourse.bass as bass  # noqa: F401  (import keeps bass registered)
import concourse.tile as tile
from concourse import bacc, mybir
from concourse.alu_op_type import AluOpType
from concourse.bass_utils import run_bass_kernel_spmd

F32 = mybir.dt.float32
BF16 = mybir.dt.bfloat16
FP8 = mybir.dt.float8e4
AF = mybir.ActivationFunctionType
OP = AluOpType
DR = mybir.MatmulPerfMode.DoubleRow
AXX = mybir.AxisListType.X

B, C, H, W = 2, 512, 64, 64
HW = H * W          # 4096 spatial tokens
P = 128             # partitions
CT = C // P         # 4 channel tiles
PT = CT // 2        # 2 channel pair-tiles
NCORES = 8
QN = HW // 4        # 1024 queries per core
CHW = 512           # token chunk width
JT = HW // P        # 32 key tiles
JJ = JT // 2        # 16 key pair-tiles
EPS = 1e-6
SCALE = float(C) ** -0.5
SH = 2.5            # constant logit shift (softmax-invariant; fp8 range)


def _build_body(nc, tc, ctx, d):
    x8_d = d["x8"]
    xb_d = d["xb"]
    y_d = d["y"]

    cpool = ctx.enter_context(tc.tile_pool(name="const", bufs=1))
    ppool = ctx.enter_context(tc.tile_pool(name="persist", bufs=1))
    spool = ctx.enter_context(tc.tile_pool(name="stream", bufs=2))
    smpool = ctx.enter_context(tc.tile_pool(name="small", bufs=1))
    qpool = ctx.enter_context(tc.tile_pool(name="psum", bufs=3, space="PSUM"))

    # DMA queue assignment: ScalarE's instruction queue must stay clear for
    # the phase-1 Square stream (a DMA descriptor-gen blocked on ring space
    # would stall everything behind it), so ScalarE only issues a share of
    # the x8 transfers (needed earliest); weights, consts and the low-
    # priority bf16 x ride on gpsimd/sync.
    bulk_engines = [nc.gpsimd, nc.sync]

    # small constants first (tiny; must not sit behind bulk transfers)
    ind8 = cpool.tile([P, 2, 16], FP8, tag="ind8")
    nc.gpsimd.dma_start(ind8[:], d["ind8"][:])
    indTA = cpool.tile([16, P], F32, tag="indTA")
    nc.sync.dma_start(indTA[:], d["indTA"][:])
    indTB = cpool.tile([16, P], F32, tag="indTB")
    nc.sync.dma_start(indTB[:], d["indTB"][:])
    chv24 = cpool.tile([P, 6 * CT], F32, tag="chv24")
    nc.gpsimd.dma_start(chv24[:], d["chv"][:])

    # ---- phase 1: x arrives fp8 in channel-pair layout (half-tile
    # transfers with 2KB contiguous rows, earliest tokens first) and is the
    # matmul operand for the whole kernel
    x8 = [ppool.tile([P, 2, HW], FP8, tag=f"x8{t}", name=f"x8{t}")
          for t in range(PT)]
    dma3 = [nc.gpsimd, nc.scalar, nc.sync]
    HN = HW // 2
    for hf in range(2):
        for t in range(PT):
            eng = dma3[(hf * PT + t) % 3]
            eng.dma_start(x8[t][:, :, hf * HN:(hf + 1) * HN], x8_d[t, hf])
    # chv columns per tile t at 6*t+j: gamma, beta, bq, bk, bv, bp
    chv3 = chv24.rearrange("p (t six) -> p t six", six=6)

    def chvcol(t, c):
        return chv24[:, 6 * t + c:6 * t + c + 1]
    # bulk weights, in consumption order, on the gpsimd/sync queues
    wts = {}
    for wi, name in enumerate(("wkT", "wvT", "wqT")):
        wts[name] = []
        for t in range(CT):
            w = cpool.tile([P, C], BF16, tag=f"{name}{t}", name=f"{name}{t}")
            bulk_engines[(wi * CT + t) % 2].dma_start(w[:], d[name][t])
            wts[name].append(w)
    # raw fp8 weight pairs for the (low-precision) bias contracts
    wr8 = {}
    for wi, name in enumerate(("wkT", "wvT", "wqT")):
        wr8[name] = []
        for t in range(PT):
            w = cpool.tile([P, 2, C], FP8, tag=f"r{name}{t}",
                           name=f"r{name}{t}")
            bulk_engines[(wi * PT + t) % 2].dma_start(w[:], d["wr8"][wi, t])
            wr8[name].append(w)
    wp8 = []
    for t in range(PT):
        w = cpool.tile([P, 2, C], FP8, tag=f"wp8{t}", name=f"wp8{t}")
        bulk_engines[t % 2].dma_start(w[:], d["wp8"][t])
        wp8.append(w)
    # fp8 delta of x (x ~= x8 + d8, residual error below bf16 level) for
    # the residual path only -- needed from the xyb precompute on, so it
    # rides last on the gpsimd/sync queues and is half the bf16 bytes
    d8_sb = [ppool.tile([P, 2, HW], FP8, tag=f"d8{t}", name=f"d8{t}")
             for t in range(PT)]
    for t in range(PT):
        for hf in range(2):
            bulk_engines[(t * 2 + hf) % 2].dma_start(
                d8_sb[t][:, :, hf * HN:(hf + 1) * HN], xb_d[t, hf])

    ones8 = cpool.tile([P, 2, 16], FP8, tag="ones8")
    nc.vector.memset(ones8[:], 1.0)
    ones_row = smpool.tile([1, P], BF16, tag="onesr")
    nc.vector.memset(ones_row[:], 1.0)
    epst16 = smpool.tile([16, 2], F32, tag="eps")
    nc.vector.memset(epst16[:], EPS)
    ebias = smpool.tile([P, 1], F32, tag="ebias")
    nc.vector.memset(ebias[:], -SH)
    i16hw = smpool.tile([16, 1], F32, tag="i16hw")
    nc.vector.memset(i16hw[:], 1.0 / (16.0 * HW))
    dumt = smpool.tile([16, 1], F32, tag="dumt")

    # GroupNorm stats on the otherwise-idle PE: per 512-token chunk, a
    # DoubleRow matmul with a group-indicator lhsT accumulates per-group
    # token-position sums in PSUM; a second accumulator consumes x^2 chunks
    # produced by a square pass split across ScalarE (Square activation)
    # and DVE (tensor_tensor mult). Group g of pair-tile T sits at PSUM
    # row 8*parity + group-within-parity.
    psgS = [qpool.tile([16, CHW], F32, tag=f"po{T}", bufs=1, name=f"psgS{T}")
            for T in range(PT)]
    psgQ = [qpool.tile([16, CHW], F32, tag=f"po{2 + T}", bufs=1,
                       name=f"psgQ{T}") for T in range(PT)]
    sqjob = 0
    for qtr in range(4):
        for T in range(PT):
            for h in range(2):
                ch = 2 * qtr + h
                sl = slice(ch * CHW, (ch + 1) * CHW)
                nc.tensor.matmul(psgS[T][:], ind8[:], x8[T][:, :, sl],
                                 start=(ch == 0), stop=(ch == 7),
                                 perf_mode=DR)
        for T in range(PT):
            for h in range(2):
                ch = 2 * qtr + h
                sl = slice(ch * CHW, (ch + 1) * CHW)
                # square pass split three ways (GpSimd shares the load; its
                # queue frees up once its DMA descriptor-gens are done)
                eng = sqjob % 3
                sqjob += 1
                xq = spool.tile([P, 2, CHW], FP8,
                                tag=f"xsq{eng}", bufs=3)
                if eng == 0:
                    nc.scalar.activation(xq[:], x8[T][:, :, sl], AF.Square)
                elif eng == 1:
                    nc.vector.tensor_tensor(xq[:], x8[T][:, :, sl],
                                            x8[T][:, :, sl], op=OP.mult)
                else:
                    nc.gpsimd.tensor_tensor(xq[:], x8[T][:, :, sl],
                                            x8[T][:, :, sl], op=OP.mult)
                nc.tensor.matmul(psgQ[T][:], ind8[:], xq[:],
                                 start=(ch == 0), stop=(ch == 7),
                                 perf_mode=DR)
    # preload the Sqrt table right behind the last Square (the combine
    # chain below hides the 1.3us load)
    nc.scalar.activation(dumt[:], epst16[:, 0:1], AF.Sqrt)
    s2 = smpool.tile([16, PT, 2], F32, tag="s2")
    # S-reduces first: psgS closes with the last x8 chunk, well before the
    # square pass drains, so they run under the square tail
    for T in range(PT):
        nc.vector.tensor_reduce(s2[:, T, 0:1], psgS[T][:], AXX, OP.add)
    for T in range(PT):
        nc.vector.tensor_reduce(s2[:, T, 1:2], psgQ[T][:], AXX, OP.add)
    s2m = smpool.tile([16, PT, 2], F32, tag="s2m")   # [mu, E[x^2]] per T
    nc.vector.tensor_scalar(s2m[:], s2[:], i16hw[:, 0:1], None, OP.mult)
    msq = smpool.tile([16, PT], F32, tag="msq")
    nc.vector.tensor_tensor(msq[:], s2m[:, :, 0], s2m[:, :, 0], op=OP.mult)
    varg = smpool.tile([16, PT], F32, tag="varg")
    nc.vector.tensor_tensor(varg[:], s2m[:, :, 1], msq[:], op=OP.subtract)
    stdg = smpool.tile([16, PT], F32, tag="stdg")
    nc.scalar.activation(stdg[:], varg[:], AF.Sqrt, bias=epst16[:, 0:1])
    # preload the Exp table right after the last Sqrt (copies in between
    # are table-neutral), so phase 3's first exp starts without a reload
    nc.scalar.activation(dumt[:], stdg[:, 0:1], AF.Exp)
    # interleave (mu_T, rstd_T) columns and broadcast groups->channels with
    # two [K=16, M=128, N=4] indicator matmuls (one per channel parity)
    mr = smpool.tile([16, PT, 2], F32, tag="mr")
    nc.vector.tensor_copy(mr[:, :, 0], s2m[:, :, 0])
    nc.vector.reciprocal(mr[:, :, 1], stdg[:])
    cbA = qpool.tile([P, 2 * PT], F32, tag="pa")
    nc.tensor.matmul(cbA[:], indTA[:], mr[:], start=True, stop=True)
    cbB = qpool.tile([P, 2 * PT], F32, tag="pa")
    nc.tensor.matmul(cbB[:], indTB[:], mr[:], start=True, stop=True)
    # cb4[p, t, {mu,rstd}] with t = 2T + parity
    cb4 = smpool.tile([P, CT, 2], F32, tag="cb4")
    cb4v = cb4.rearrange("p (T e) two -> p T e two", e=2)
    nc.vector.tensor_copy(cb4v[:, :, 0, :], cbA[:])
    nc.vector.tensor_copy(cb4v[:, :, 1, :], cbB[:])

    # per-channel Scale a / Bias b, vectorized across all 4 tiles via
    # strided views (one wide DVE op instead of one per tile)
    ab = ppool.tile([P, 2, CT], F32, tag="ab")   # [:,0,t]=a_t  [:,1,t]=b_t
    nc.vector.tensor_tensor(ab[:, 0, :], cb4[:, :, 1], chv3[:, :, 0],
                            op=OP.mult)
    tmpb = smpool.tile([P, CT], F32, tag="tmpb", bufs=1)
    nc.vector.tensor_tensor(tmpb[:], cb4[:, :, 0], ab[:, 0, :], op=OP.mult)
    nc.vector.tensor_tensor(ab[:, 1, :], chv3[:, :, 1], tmpb[:],
                            op=OP.subtract)
    # b as fp8 pairs for the (tiny-magnitude) bias contracts
    bv8 = [ppool.tile([P, 2, 16], FP8, tag=f"bv8{T}", name=f"bv8{T}")
           for T in range(PT)]
    for t in range(CT):
        nc.vector.tensor_copy(bv8[t // 2][:, t % 2, 0:1], ab[:, 1, t:t + 1])
        nc.vector.tensor_copy(bv8[t // 2][:, t % 2, 1:2], ab[:, 1, t:t + 1])
    sbts = [ab[:, 0, t:t + 1] for t in range(CT)]

    # GroupNorm scale folded into fp8 PAIR COPIES of the projection weights
    # (the raw bf16 tiles stay live for the bias contracts interleaved into
    # phase 2 below); split DVE/ScalarE (activation-Copy is table-neutral),
    # wk first so the first k-projection chunk is gated as early as possible
    ws8 = {}
    for name in ("wkT", "wvT", "wqT"):
        ws8[name] = []
        for t in range(PT):
            w = cpool.tile([P, 2, C], FP8, tag=f"s{name}{t}",
                           name=f"s{name}{t}")
            nc.vector.tensor_scalar_mul(w[:, 0, :], wts[name][2 * t][:],
                                        sbts[2 * t])
            nc.scalar.activation(w[:, 1, :], wts[name][2 * t + 1][:],
                                 AF.Copy, scale=sbts[2 * t + 1])
            ws8[name].append(w)

    # ---- bias-term constants from RAW weights (tiny N=2 matmuls);
    # emitted one output-tile group at a time, interleaved into phase 2's
    # first chunks so they never head-of-line block the projections ----
    #   qb[o] = sum_c wq[o,c] b[c] + bq    (added at the q PSUM->SBUF move)
    #   kb[o] = likewise with bk
    #   vb8[c] = sum_cin wv[c,cin] b[cin] + bv   (rides softmax into yb)
    #   yb[o] = sum_c wp[o,c] vb8[c] + bp        (y epilogue constant)
    vb8 = [ppool.tile([P, 2, 16], FP8, tag=f"vb8{t}", name=f"vb8{t}")
           for t in range(PT)]

    def bias_ct(wname, ot, outdt, addcol, tagp):
        pb = qpool.tile([P, 2], F32, tag="pa", name="pb")
        for t in range(PT):
            nc.tensor.matmul(pb[:], wr8[wname][t][:, :, ot * P:(ot + 1) * P],
                             bv8[t][:, :, 0:2], start=(t == 0),
                             stop=(t == PT - 1), perf_mode=DR)
        if outdt == F32:
            ob = ppool.tile([P, 1], F32, tag=f"{tagp}{ot}", name=f"{tagp}{ot}")
            nc.vector.tensor_scalar(ob[:], pb[:, 0:1], chvcol(ot, addcol),
                                    None, OP.add)
            return ob
        # fp8 pair column for the yb contract
        tf = smpool.tile([P, 1], F32, tag="tf", bufs=2)
        nc.vector.tensor_scalar(tf[:], pb[:, 0:1], chvcol(ot, addcol),
                                None, OP.add)
        nc.vector.tensor_copy(vb8[ot // 2][:, ot % 2, 0:1], tf[:])
        nc.vector.tensor_copy(vb8[ot // 2][:, ot % 2, 1:2], tf[:])
        return None

    def yb_ct(ot):
        pb = qpool.tile([P, 2], F32, tag="pa", name="pb")
        for t in range(PT):
            nc.tensor.matmul(pb[:], wp8[t][:, :, ot * P:(ot + 1) * P],
                             vb8[t][:, :, 0:2], start=(t == 0),
                             stop=(t == PT - 1), perf_mode=DR)
        ob = ppool.tile([P, 1], F32, tag=f"yb{ot}", name=f"yb{ot}")
        nc.vector.tensor_scalar(ob[:], pb[:, 0:1], chvcol(ot, 5),
                                None, OP.add)
        return ob

    # ---- persistent attention operands (all fp8 pairs) ----
    k8 = [ppool.tile([P, 2, HW], FP8, tag=f"k8{t}", name=f"k8{t}")
          for t in range(PT)]
    q8 = [ppool.tile([P, 2, QN], FP8, tag=f"q8{t}", name=f"q8{t}")
          for t in range(PT)]
    vT8 = [ppool.tile([P, 2, C], FP8, tag=f"vT8{j}", name=f"vT8{j}")
           for j in range(JJ)]

    # ---- phase 2: q/k/v projections straight from resident fp8 x; the
    # bias contracts ride along inside chunks 0-1 ----
    kb, qb, yb = [None] * CT, [None] * CT, [None] * CT
    p2tags = ["pa", "po0", "po1", "po2", "po3"]
    p2cnt = [0]

    def p2psum():
        tag = p2tags[p2cnt[0] % len(p2tags)]
        p2cnt[0] += 1
        return qpool.tile([P, CHW], F32, tag=tag,
                          bufs=(3 if tag == "pa" else 1), name="p2")

    for ch in range(HW // CHW):
        sl = slice(ch * CHW, (ch + 1) * CHW)
        for ot in range(CT):
            pk = p2psum()
            for t in range(PT):
                nc.tensor.matmul(pk[:], ws8["wkT"][t][:, :, ot * P:(ot + 1) * P],
                                 x8[t][:, :, sl], start=(t == 0),
                                 stop=(t == PT - 1), perf_mode=DR)
            if ch == 0:
                kb[ot] = bias_ct("wkT", ot, F32, 3, "kb")
            nc.vector.tensor_scalar(k8[ot // 2][:, ot % 2, sl], pk[:],
                                    kb[ot][:, 0:1], None, OP.add)
        for nt in range(CT):
            jt = ch * CT + nt
            pv = p2psum()
            for t in range(PT):
                nc.tensor.matmul(pv[:], x8[t][:, :, ch * CHW + nt * P:
                                              ch * CHW + (nt + 1) * P],
                                 ws8["wvT"][t][:], start=(t == 0),
                                 stop=(t == PT - 1), perf_mode=DR)
            if ch == 0:
                bias_ct("wvT", nt, FP8, 4, "vbt")
            elif ch == 1:
                yb[nt] = yb_ct(nt)
            nc.scalar.copy(vT8[jt // 2][:, jt % 2, :], pv[:])
        if ch * CHW < QN:
            for ot in range(CT):
                pq = p2psum()
                for t in range(PT):
                    nc.tensor.matmul(pq[:],
                                     ws8["wqT"][t][:, :, ot * P:(ot + 1) * P],
                                     x8[t][:, :, sl], start=(t == 0),
                                     stop=(t == PT - 1), perf_mode=DR)
                if ch == 0:
                    qb[ot] = bias_ct("wqT", ot, F32, 2, "qb")
                nc.vector.tensor_scalar(q8[ot // 2][:, ot % 2, sl], pq[:],
                                        qb[ot][:, 0:1], None, OP.add)

    # x + yb = (x8 + d8) + yb, precomputed off the critical path for the y
    # epilogue
    xyb = [[None] * CT for _ in range(2)]
    for ih in range(2):
        for ot in range(CT):
            sl = slice(ih * CHW, (ih + 1) * CHW)
            T, e = ot // 2, ot % 2
            xt = ppool.tile([P, CHW], F32, tag=f"xyb{ih}{ot}",
                            name=f"xyb{ih}{ot}")
            nc.vector.tensor_scalar(xt[:], d8_sb[T][:, e, sl],
                                    yb[ot][:, 0:1], None, OP.add)
            nc.vector.tensor_tensor(xt[:], xt[:], x8[T][:, e, sl], op=OP.add)
            xyb[ih][ot] = xt

    # ---- phase 3: attention, per query half ----
    def mk_pr():
        return qpool.tile([1, CHW], F32, tag="pr", bufs=1, name="pr")

    def mk_po():
        return [qpool.tile([P, CHW], F32, tag=f"po{t}", name=f"po{t}", bufs=1)
                for t in range(CT)]

    def sc_exp(ih, j):
        isl = slice(ih * CHW, (ih + 1) * CHW)
        ps_ = qpool.tile([P, CHW], F32, tag="pa", name="ps")
        for t in range(PT):
            nc.tensor.matmul(ps_[:], k8[t][:, :, j * P:(j + 1) * P],
                             q8[t][:, :, isl], start=(t == 0),
                             stop=(t == PT - 1), perf_mode=DR)
        pT = spool.tile([P, 2, CHW], FP8, tag="pT", bufs=12, name="pT") \
            if j % 2 == 0 else None
        return ps_, pT

    def exp_into(pair, par, ps_):
        nc.scalar.activation(pair[:, par, :], ps_[:], AF.Exp, scale=SCALE,
                             bias=ebias[:, 0:1])

    def av_only(po, jj, pair):
        for t in range(CT):
            nc.tensor.matmul(po[t][:], vT8[jj][:, :, t * P:(t + 1) * P],
                             pair[:], start=(jj == 0), stop=(jj == JJ - 1),
                             perf_mode=DR)

    def rowsum_only(pr, jj, pair):
        nc.tensor.matmul(pr[:], ones8[:, :, 0:1], pair[:],
                         start=(jj == 0), stop=(jj == JJ - 1), perf_mode=DR)

    def tail_and_y(pr, po, ih):
        # 1/r chain first: pr closed early (rowsum bursts), so DVE computes
        # rinv/rbb and the broadcast matmul lands BEFORE the last AV
        # finishes; the softmax normalization folds into the fp8 'has'
        # tiles (has = po * (1/r)), so the epilogue is a single add
        rinv = smpool.tile([1, CHW], F32, tag="rinv", bufs=2)
        nc.vector.reciprocal_approx_fast(rinv[:], pr[:])
        rbb = smpool.tile([1, CHW], BF16, tag="rbb", bufs=2)
        nc.vector.tensor_copy(rbb[:], rinv[:])
        rb = spool.tile([P, CHW], F32, tag="rb", bufs=2)
        prb = qpool.tile([P, CHW], F32, tag="pa")
        nc.tensor.matmul(prb[:], ones_row[:], rbb[:], start=True, stop=True)
        # rb copy on ScalarE (Copy is table-neutral): frees DVE to start the
        # has multiplies the moment rb lands
        nc.scalar.copy(rb[:], prb[:])
        has = []
        for t in range(PT):
            ha = spool.tile([P, 2, CHW], FP8, tag=f"hx{t}", bufs=2)
            nc.vector.tensor_tensor(ha[:, 0, :], po[2 * t][:], rb[:],
                                    op=OP.mult)
            nc.vector.tensor_tensor(ha[:, 1, :], po[2 * t + 1][:], rb[:],
                                    op=OP.mult)
            has.append(ha)
        for ot in range(CT):
            # reuse the freed po slot: the pa slots stay available for the
            # next half's score pipeline even while the 1/r chain lags
            py = qpool.tile([P, CHW], F32, tag=f"po{ot}", name="py", bufs=1)
            for t in range(PT):
                nc.tensor.matmul(py[:], wp8[t][:, :, ot * P:(ot + 1) * P],
                                 has[t][:], start=(t == 0),
                                 stop=(t == PT - 1), perf_mode=DR)
            # column-split epilogue: each half fires its y DMA as soon as
            # its DVE add is done, hiding the per-transfer DMA latency
            yt = spool.tile([P, CHW], F32, tag="yt", bufs=3)
            for hc in range(2):
                cs = slice(hc * (CHW // 2), (hc + 1) * (CHW // 2))
                nc.vector.tensor_tensor(yt[:, cs], py[:, cs],
                                        xyb[ih][ot][:, cs], op=OP.add)
                # y spread over all 3 DMA queues (ScalarE is idle and its
                # ring long drained by the tails)
                dma3[(2 * ot + hc) % 3].dma_start(
                    y_d[ot, :, ih * CHW + hc * (CHW // 2):
                        ih * CHW + (hc + 1) * (CHW // 2)], yt[:, cs])

    # scores/exp run LA jj-pairs ahead of the AV that consumes the pair:
    # by the time the PE reaches each consumer, the exp's (late-posting)
    # completion semaphore is stale and the PE never waits on ScalarE. The
    # next half's first KPRE pair groups are emitted into the drain/tail
    # window so the PE never idles across halves.
    LA, KPRE = 2, 3
    pr0 = mk_pr()
    po0 = mk_po()
    pr1 = mk_pr()
    pairs0, pre = {}, {}
    for it in range(JJ + KPRE):
        if it < JJ:
            ps0, pair = sc_exp(0, 2 * it)
            exp_into(pair, 0, ps0)
            ps1, _ = sc_exp(0, 2 * it + 1)
            exp_into(pair, 1, ps1)
            pairs0[it] = pair
        elif it - JJ < KPRE:
            jjp = it - JJ
            ps0, pair = sc_exp(1, 2 * jjp)
            exp_into(pair, 0, ps0)
            ps1, _ = sc_exp(1, 2 * jjp + 1)
            exp_into(pair, 1, ps1)
            pre[jjp] = pair
        # rowsums in bursts of 8 pairs: the M=1 matmul costs PE reconfig on
        # each entry/exit, so amortize it; the last burst lands before the
        # last AV so the 1/r chain overlaps it
        if it in (JJ // 2, JJ):
            for jp in range(it - JJ // 2, it):
                rowsum_only(pr0, jp, pairs0[jp])
        ja = it - LA
        if 0 <= ja < JJ:
            av_only(po0, ja, pairs0[ja])
    tail_and_y(pr0, po0, 0)
    po1 = mk_po()
    for it in range(JJ):
        js = it + KPRE
        if js < JJ:
            ps0, pair = sc_exp(1, 2 * js)
            exp_into(pair, 0, ps0)
            ps1, _ = sc_exp(1, 2 * js + 1)
            exp_into(pair, 1, ps1)
            pre[js] = pair
        if it in (JJ // 2 - 1, JJ - 1):
            for jp in range(it - JJ // 2 + 1, it + 1):
                rowsum_only(pr1, jp, pre[jp])
        av_only(po1, it, pre[it])
    tail_and_y(pr1, po1, 1)


def build_module():
    nc = bacc.Bacc("TRN2", target_bir_lowering=False, debug=False,
                   num_devices=NCORES)
    d = {
        "x8": nc.dram_tensor("x8", [PT, 2, P, 2, HW // 2], FP8,
                             kind="ExternalInput").ap(),
        "xb": nc.dram_tensor("xb", [PT, 2, P, 2, HW // 2], FP8,
                             kind="ExternalInput").ap(),
        "wqT": nc.dram_tensor("wqT", [CT, P, C], BF16, kind="ExternalInput").ap(),
        "wkT": nc.dram_tensor("wkT", [CT, P, C], BF16, kind="ExternalInput").ap(),
        "wvT": nc.dram_tensor("wvT", [CT, P, C], BF16, kind="ExternalInput").ap(),
        "wp8": nc.dram_tensor("wp8", [PT, P, 2, C], FP8,
                              kind="ExternalInput").ap(),
        "wr8": nc.dram_tensor("wr8", [3, PT, P, 2, C], FP8,
                              kind="ExternalInput").ap(),
        "chv": nc.dram_tensor("chv", [P, 6 * CT], F32, kind="ExternalInput").ap(),
        "ind8": nc.dram_tensor("ind8", [P, 2, 16], FP8,
                               kind="ExternalInput").ap(),
        "indTA": nc.dram_tensor("indTA", [16, P], F32,
                                kind="ExternalInput").ap(),
        "indTB": nc.dram_tensor("indTB", [16, P], F32,
                                kind="ExternalInput").ap(),
        "y": nc.dram_tensor("y", [CT, P, QN], F32, kind="ExternalOutput").ap(),
    }
    with tile.TileContext(nc) as tc, ExitStack() as ctx:
        _build_body(nc, tc, ctx, d)
    nc.compile()
    return nc


_CACHE = {}


def _get_nc():
    if "nc" not in _CACHE:
        _CACHE["nc"] = build_module()
    return _CACHE["nc"]


def _shared_inputs(gamma, beta, wq, bq, wk, bk, wv, bv, wp, bp):
    import ml_dtypes

    def wT(w):
        wt = np.ascontiguousarray(np.asarray(w, np.float32).T)
        return wt.reshape(CT, P, C).astype(ml_dtypes.bfloat16)

    def w8(w):
        wt = np.asarray(w, np.float32).T.reshape(PT, 2, P, C)
        return np.ascontiguousarray(wt.transpose(0, 2, 1, 3)).astype(
            ml_dtypes.float8_e4m3)

    wp8 = w8(wp)
    wr8 = np.stack([w8(wk), w8(wv), w8(wq)])

    # group indicator for the PE stats matmuls: partition p / parity e of a
    # pair-tile belongs to group row 8*e + p//16
    ind8 = np.zeros((P, 2, 16), np.float32)
    for p in range(P):
        for e in range(2):
            ind8[p, e, 8 * e + p // 16] = 1.0
    # broadcast-back indicators (per parity): group row -> partition
    indTA = np.zeros((16, P), np.float32)
    indTB = np.zeros((16, P), np.float32)
    for p in range(P):
        indTA[p // 16, p] = 1.0
        indTB[8 + p // 16, p] = 1.0
    chv = np.stack([np.asarray(a, np.float32)
                    for a in (gamma, beta, bq, bk, bv, bp)],
                   axis=1).reshape(CT, P, 6).transpose(1, 0, 2).reshape(P, 24)
    return {
        "wqT": wT(wq), "wkT": wT(wk), "wvT": wT(wv), "wp8": wp8,
        "wr8": wr8,
        "chv": np.ascontiguousarray(chv),
        "ind8": ind8.astype(ml_dtypes.float8_e4m3),
        "indTA": indTA, "indTB": indTB,
    }


def make_in_maps(x, gamma, beta, wq, bq, wk, bk, wv, bv, wp, bp):
    import ml_dtypes

    shared = _shared_inputs(gamma, beta, wq, bq, wk, bk, wv, bv, wp, bp)
    xf = np.asarray(x, np.float32).reshape(B, C, HW)
    in_maps = []
    for core in range(NCORES):
        b, qc = divmod(core, NCORES // B)
        xr = np.roll(xf[b], -qc * QN, axis=1)          # [C, HW]
        # fp8 channel-pair layout [T, half, p, e, m]: c = 256T+128e+p
        xp = np.ascontiguousarray(
            xr.reshape(PT, 2, P, 2, HW // 2).transpose(0, 3, 2, 1, 4))
        x8 = xp.astype(ml_dtypes.float8_e4m3)
        d8 = (xp - x8.astype(np.float32)).astype(ml_dtypes.float8_e4m3)
        m = dict(shared)
        m["x8"] = x8
        m["xb"] = d8
        in_maps.append(m)
    return in_maps


def assemble_output(results):
    out = np.empty((B, C, HW), np.float32)
    for core in range(NCORES):
        b, qc = divmod(core, NCORES // B)
        y = np.asarray(results[core]["y"]).reshape(C, QN)
        out[b, :, qc * QN:(qc + 1) * QN] = y
    return out.reshape(B, C, H, W)


def kernel(x, gamma, beta, wq, bq, wk, bk, wv, bv, wp, bp):
    nc = _get_nc()
    in_maps = make_in_maps(x, gamma, beta, wq, bq, wk, bk, wv, bv, wp, bp)
    res = run_bass_kernel_spmd(nc, in_maps, list(range(NCORES)))
    return assemble_output(res.results)


# revision 26
# speedup vs baseline: 1.0415x; 1.0415x over previous
"""Trainium2 Bass kernel for nn_AttentionBlock (GroupNorm + single-head spatial
self-attention + residual) on 8 NeuronCores.

Sharding: data-parallel over batch (2) x sequence-parallel over the query
dimension (4 chunks of 1024 of the 4096 spatial tokens). Each core gets the
full image of its batch element, ROTATED so its query chunk sits at token 0
(GroupNorm stats, key/value sets and softmax sums are permutation-invariant
over tokens, so rotation lets all 8 cores run the identical SPMD program).

v2: every large matmul runs in fp8e4m3 with perf_mode=DoubleRow (two K=128
contraction tiles fused per instruction, ~1.44x PE throughput at FD>=512).
All pairs are [128, 2, N] tiles pairing adjacent 128-blocks of the
contraction dim. Measured end-to-end rel err ~9e-3 vs the 2e-2 gate.

  x ships as fp8 channel-pairs (2MB) and is the matmul operand for q/k/v;
  a second bf16 copy of x arrives later (DMA idle mid-kernel) only for the
  residual. GroupNorm stats come from the fp8 x via DVE free-dim reduces
  (sums) + ScalarE Square activations with accum_out (sumsq) streamed
  behind the DMAs; group-combine via tiny PE indicator matmuls as before.
  The GroupNorm scale a[c] folds into fp8 copies of wq/wk/wv (bf16 raw
  weights ship for the tiny bias contracts); the shift b[c] becomes
  per-output-channel constants (rows of w.b) as in v1.

  scores: sT[j,i] = sum_c k[c,j] q[c,i] as 2 DoubleRow matmuls (c-pairs);
  exp on ScalarE with a constant logit shift of -2.5 (softmax is shift
  invariant; the shift keeps exp under fp8 max 240; logit max is ~7.2)
  writing fp8 straight into j-pair tiles. AV + rowsum consume the pairs
  with DoubleRow (vT j-pairs / fp8 ones). Softmax normalization is
  deferred past the wp projection: po -> fp8 'has' pairs scaled by 1/16
  (keeps po under fp8 range; the x16 folds into the 1/r row broadcast,
  whose ones-row carries value 16). Final y = py*(16/r) + (x + yb).
"""

import sys
from contextlib import ExitStack

if "/opt/trn_rl_repo" not in sys.path:
    sys.path.insert(0, "/opt/trn_rl_repo")

import numpy as np

import concourse.bass as bass  # noqa: F401  (import keeps bass registered)
import concourse.tile as tile
from concourse import bacc, mybir
from concourse.alu_op_type import AluOpType
from concourse.bass_utils import run_bass_kernel_spmd

F32 = mybir.dt.float32
BF16 = mybir.dt.bfloat16
FP8 = mybir.dt.float8e4
AF = mybir.ActivationFunctionType
OP = AluOpType
DR = mybir.MatmulPerfMode.DoubleRow
AXX = mybir.AxisListType.X

B, C, H, W = 2, 512, 64, 64
HW = H * W          # 4096 spatial tokens
P = 128             # partitions
CT = C // P         # 4 channel tiles
PT = CT // 2        # 2 channel pair-tiles
NCORES = 8
QN = HW // 4        # 1024 queries per core
CHW = 512           # token chunk width
JT = HW // P        # 32 key tiles
JJ = JT // 2        # 16 key pair-tiles
EPS = 1e-6
SCALE = float(C) ** -0.5
SH = 2.5            # constant logit shift (softmax-invariant; fp8 range)


def _build_body(nc, tc, ctx, d):
    x8_d = d["x8"]
    xb_d = d["xb"]
    y_d = d["y"]

    cpool = ctx.enter_context(tc.tile_pool(name="const", bufs=1))
    ppool = ctx.enter_context(tc.tile_pool(name="persist", bufs=1))
    spool = ctx.enter_context(tc.tile_pool(name="stream", bufs=2))
    smpool = ctx.enter_context(tc.tile_pool(name="small", bufs=1))
    qpool = ctx.enter_context(tc.tile_pool(name="psum", bufs=3, space="PSUM"))

    # DMA queue assignment: ScalarE's instruction queue must stay clear for
    # the phase-1 Square stream (a DMA descriptor-gen blocked on ring space
    # would stall everything behind it), so ScalarE only issues a share of
    # the x8 transfers (needed earliest); weights, consts and the low-
    # priority bf16 x ride on gpsimd/sync.
    bulk_engines = [nc.gpsimd, nc.sync]

    # small constants first (tiny; must not sit behind bulk transfers)
    ind8 = cpool.tile([P, 2, 16], FP8, tag="ind8")
    nc.gpsimd.dma_start(ind8[:], d["ind8"][:])
    indTA = cpool.tile([16, P], F32, tag="indTA")
    nc.sync.dma_start(indTA[:], d["indTA"][:])
    indTB = cpool.tile([16, P], F32, tag="indTB")
    nc.sync.dma_start(indTB[:], d["indTB"][:])
    chv24 = cpool.tile([P, 6 * CT], F32, tag="chv24")
    nc.gpsimd.dma_start(chv24[:], d["chv"][:])

    # ---- phase 1: x arrives fp8 in channel-pair layout (half-tile
    # transfers with 2KB contiguous rows, earliest tokens first) and is the
    # matmul operand for the whole kernel
    x8 = [ppool.tile([P, 2, HW], FP8, tag=f"x8{t}", name=f"x8{t}")
          for t in range(PT)]
    dma3 = [nc.gpsimd, nc.scalar, nc.sync]
    HN = HW // 2
    for hf in range(2):
        for t in range(PT):
            eng = dma3[(hf * PT + t) % 3]
            eng.dma_start(x8[t][:, :, hf * HN:(hf + 1) * HN], x8_d[t, hf])
    # chv columns per tile t at 6*t+j: gamma, beta, bq, bk, bv, bp
    chv3 = chv24.rearrange("p (t six) -> p t six", six=6)

    def chvcol(t, c):
        return chv24[:, 6 * t + c:6 * t + c + 1]
    # bulk weights, in consumption order, on the gpsimd/sync queues
    wts = {}
    for wi, name in enumerate(("wkT", "wvT", "wqT")):
        wts[name] = []
        for t in range(CT):
            w = cpool.tile([P, C], BF16, tag=f"{name}{t}", name=f"{name}{t}")
            bulk_engines[(wi * CT + t) % 2].dma_start(w[:], d[name][t])
            wts[name].append(w)
    # raw fp8 weight pairs for the (low-precision) bias contracts
    wr8 = {}
    for wi, name in enumerate(("wkT", "wvT", "wqT")):
        wr8[name] = []
        for t in range(PT):
            w = cpool.tile([P, 2, C], FP8, tag=f"r{name}{t}",
                           name=f"r{name}{t}")
            bulk_engines[(wi * PT + t) % 2].dma_start(w[:], d["wr8"][wi, t])
            wr8[name].append(w)
    wp8 = []
    for t in range(PT):
        w = cpool.tile([P, 2, C], FP8, tag=f"wp8{t}", name=f"wp8{t}")
        bulk_engines[t % 2].dma_start(w[:], d["wp8"][t])
        wp8.append(w)
    # fp8 delta of x (x ~= x8 + d8, residual error below bf16 level) for
    # the residual path only -- needed from the xyb precompute on, so it
    # rides last on the gpsimd/sync queues and is half the bf16 bytes
    d8_sb = [ppool.tile([P, 2, HW], FP8, tag=f"d8{t}", name=f"d8{t}")
             for t in range(PT)]
    for t in range(PT):
        for hf in range(2):
            bulk_engines[(t * 2 + hf) % 2].dma_start(
                d8_sb[t][:, :, hf * HN:(hf + 1) * HN], xb_d[t, hf])

    ones8 = cpool.tile([P, 2, 16], FP8, tag="ones8")
    nc.vector.memset(ones8[:], 1.0)
    ones_row = smpool.tile([1, P], BF16, tag="onesr")
    nc.vector.memset(ones_row[:], 1.0)
    epst16 = smpool.tile([16, 2], F32, tag="eps")
    nc.vector.memset(epst16[:], EPS)
    ebias = smpool.tile([P, 1], F32, tag="ebias")
    nc.vector.memset(ebias[:], -SH)
    i16hw = smpool.tile([16, 1], F32, tag="i16hw")
    nc.vector.memset(i16hw[:], 1.0 / (16.0 * HW))
    dumt = smpool.tile([16, 1], F32, tag="dumt")

    # GroupNorm stats on the otherwise-idle PE: per 512-token chunk, a
    # DoubleRow matmul with a group-indicator lhsT accumulates per-group
    # token-position sums in PSUM; a second accumulator consumes x^2 chunks
    # produced by a square pass split across ScalarE (Square activation)
    # and DVE (tensor_tensor mult). Group g of pair-tile T sits at PSUM
    # row 8*parity + group-within-parity.
    psgS = [qpool.tile([16, CHW], F32, tag=f"po{T}", bufs=1, name=f"psgS{T}")
            for T in range(PT)]
    psgQ = [qpool.tile([16, CHW], F32, tag=f"po{2 + T}", bufs=1,
                       name=f"psgQ{T}") for T in range(PT)]
    sqjob = 0
    for qtr in range(4):
        for T in range(PT):
            for h in range(2):
                ch = 2 * qtr + h
                sl = slice(ch * CHW, (ch + 1) * CHW)
                nc.tensor.matmul(psgS[T][:], ind8[:], x8[T][:, :, sl],
                                 start=(ch == 0), stop=(ch == 7),
                                 perf_mode=DR)
        for T in range(PT):
            for h in range(2):
                ch = 2 * qtr + h
                sl = slice(ch * CHW, (ch + 1) * CHW)
                # square pass split ScalarE/DVE (GpSimd measured 2-3x
                # slower per op and its queue is busy with DMA gens)
                eng = sqjob % 2
                sqjob += 1
                xq = spool.tile([P, 2, CHW], FP8,
                                tag=f"xsq{eng}", bufs=3)
                if eng == 0:
                    nc.scalar.activation(xq[:], x8[T][:, :, sl], AF.Square)
                else:
                    nc.vector.tensor_tensor(xq[:], x8[T][:, :, sl],
                                            x8[T][:, :, sl], op=OP.mult)
                nc.tensor.matmul(psgQ[T][:], ind8[:], xq[:],
                                 start=(ch == 0), stop=(ch == 7),
                                 perf_mode=DR)
    # preload the Sqrt table right behind the last Square (the combine
    # chain below hides the 1.3us load)
    nc.scalar.activation(dumt[:], epst16[:, 0:1], AF.Sqrt)
    s2 = smpool.tile([16, PT, 2], F32, tag="s2")
    # S-reduces first: psgS closes with the last x8 chunk, well before the
    # square pass drains, so they run under the square tail
    for T in range(PT):
        nc.vector.tensor_reduce(s2[:, T, 0:1], psgS[T][:], AXX, OP.add)
    for T in range(PT):
        nc.vector.tensor_reduce(s2[:, T, 1:2], psgQ[T][:], AXX, OP.add)
    s2m = smpool.tile([16, PT, 2], F32, tag="s2m")   # [mu, E[x^2]] per T
    nc.vector.tensor_scalar(s2m[:], s2[:], i16hw[:, 0:1], None, OP.mult)
    msq = smpool.tile([16, PT], F32, tag="msq")
    nc.vector.tensor_tensor(msq[:], s2m[:, :, 0], s2m[:, :, 0], op=OP.mult)
    varg = smpool.tile([16, PT], F32, tag="varg")
    nc.vector.tensor_tensor(varg[:], s2m[:, :, 1], msq[:], op=OP.subtract)
    stdg = smpool.tile([16, PT], F32, tag="stdg")
    nc.scalar.activation(stdg[:], varg[:], AF.Sqrt, bias=epst16[:, 0:1])
    # preload the Exp table right after the last Sqrt (copies in between
    # are table-neutral), so phase 3's first exp starts without a reload
    nc.scalar.activation(dumt[:], stdg[:, 0:1], AF.Exp)
    # interleave (mu_T, rstd_T) columns and broadcast groups->channels with
    # two [K=16, M=128, N=4] indicator matmuls (one per channel parity)
    mr = smpool.tile([16, PT, 2], F32, tag="mr")
    nc.vector.tensor_copy(mr[:, :, 0], s2m[:, :, 0])
    nc.vector.reciprocal(mr[:, :, 1], stdg[:])
    cbA = qpool.tile([P, 2 * PT], F32, tag="pa")
    nc.tensor.matmul(cbA[:], indTA[:], mr[:], start=True, stop=True)
    cbB = qpool.tile([P, 2 * PT], F32, tag="pa")
    nc.tensor.matmul(cbB[:], indTB[:], mr[:], start=True, stop=True)
    # cb4[p, t, {mu,rstd}] with t = 2T + parity
    cb4 = smpool.tile([P, CT, 2], F32, tag="cb4")
    cb4v = cb4.rearrange("p (T e) two -> p T e two", e=2)
    nc.vector.tensor_copy(cb4v[:, :, 0, :], cbA[:])
    nc.vector.tensor_copy(cb4v[:, :, 1, :], cbB[:])

    # per-channel Scale a / Bias b, vectorized across all 4 tiles via
    # strided views (one wide DVE op instead of one per tile)
    ab = ppool.tile([P, 2, CT], F32, tag="ab")   # [:,0,t]=a_t  [:,1,t]=b_t
    nc.vector.tensor_tensor(ab[:, 0, :], cb4[:, :, 1], chv3[:, :, 0],
                            op=OP.mult)
    tmpb = smpool.tile([P, CT], F32, tag="tmpb", bufs=1)
    nc.vector.tensor_tensor(tmpb[:], cb4[:, :, 0], ab[:, 0, :], op=OP.mult)
    nc.vector.tensor_tensor(ab[:, 1, :], chv3[:, :, 1], tmpb[:],
                            op=OP.subtract)
    # b as fp8 pairs for the (tiny-magnitude) bias contracts
    bv8 = [ppool.tile([P, 2, 16], FP8, tag=f"bv8{T}", name=f"bv8{T}")
           for T in range(PT)]
    for t in range(CT):
        nc.vector.tensor_copy(bv8[t // 2][:, t % 2, 0:1], ab[:, 1, t:t + 1])
        nc.vector.tensor_copy(bv8[t // 2][:, t % 2, 1:2], ab[:, 1, t:t + 1])
    sbts = [ab[:, 0, t:t + 1] for t in range(CT)]

    # GroupNorm scale folded into fp8 PAIR COPIES of the projection weights
    # (the raw bf16 tiles stay live for the bias contracts interleaved into
    # phase 2 below); split DVE/ScalarE (activation-Copy is table-neutral),
    # wk first so the first k-projection chunk is gated as early as possible
    ws8 = {}
    for name in ("wkT", "wvT", "wqT"):
        ws8[name] = []
        for t in range(PT):
            w = cpool.tile([P, 2, C], FP8, tag=f"s{name}{t}",
                           name=f"s{name}{t}")
            nc.vector.tensor_scalar_mul(w[:, 0, :], wts[name][2 * t][:],
                                        sbts[2 * t])
            nc.scalar.activation(w[:, 1, :], wts[name][2 * t + 1][:],
                                 AF.Copy, scale=sbts[2 * t + 1])
            ws8[name].append(w)

    # ---- bias-term constants from RAW weights (tiny N=2 matmuls);
    # emitted one output-tile group at a time, interleaved into phase 2's
    # first chunks so they never head-of-line block the projections ----
    #   qb[o] = sum_c wq[o,c] b[c] + bq    (added at the q PSUM->SBUF move)
    #   kb[o] = likewise with bk
    #   vb8[c] = sum_cin wv[c,cin] b[cin] + bv   (rides softmax into yb)
    #   yb[o] = sum_c wp[o,c] vb8[c] + bp        (y epilogue constant)
    vb8 = [ppool.tile([P, 2, 16], FP8, tag=f"vb8{t}", name=f"vb8{t}")
           for t in range(PT)]

    def bias_ct(wname, ot, outdt, addcol, tagp):
        pb = qpool.tile([P, 2], F32, tag="pa", name="pb")
        for t in range(PT):
            nc.tensor.matmul(pb[:], wr8[wname][t][:, :, ot * P:(ot + 1) * P],
                             bv8[t][:, :, 0:2], start=(t == 0),
                             stop=(t == PT - 1), perf_mode=DR)
        if outdt == F32:
            ob = ppool.tile([P, 1], F32, tag=f"{tagp}{ot}", name=f"{tagp}{ot}")
            nc.vector.tensor_scalar(ob[:], pb[:, 0:1], chvcol(ot, addcol),
                                    None, OP.add)
            return ob
        # fp8 pair column for the yb contract
        tf = smpool.tile([P, 1], F32, tag="tf", bufs=2)
        nc.vector.tensor_scalar(tf[:], pb[:, 0:1], chvcol(ot, addcol),
                                None, OP.add)
        nc.vector.tensor_copy(vb8[ot // 2][:, ot % 2, 0:1], tf[:])
        nc.vector.tensor_copy(vb8[ot // 2][:, ot % 2, 1:2], tf[:])
        return None

    def yb_ct(ot):
        pb = qpool.tile([P, 2], F32, tag="pa", name="pb")
        for t in range(PT):
            nc.tensor.matmul(pb[:], wp8[t][:, :, ot * P:(ot + 1) * P],
                             vb8[t][:, :, 0:2], start=(t == 0),
                             stop=(t == PT - 1), perf_mode=DR)
        ob = ppool.tile([P, 1], F32, tag=f"yb{ot}", name=f"yb{ot}")
        nc.vector.tensor_scalar(ob[:], pb[:, 0:1], chvcol(ot, 5),
                                None, OP.add)
        return ob

    # ---- persistent attention operands (all fp8 pairs) ----
    k8 = [ppool.tile([P, 2, HW], FP8, tag=f"k8{t}", name=f"k8{t}")
          for t in range(PT)]
    q8 = [ppool.tile([P, 2, QN], FP8, tag=f"q8{t}", name=f"q8{t}")
          for t in range(PT)]
    vT8 = [ppool.tile([P, 2, C], FP8, tag=f"vT8{j}", name=f"vT8{j}")
           for j in range(JJ)]

    # ---- phase 2: q/k/v projections straight from resident fp8 x; the
    # bias contracts ride along inside chunks 0-1 ----
    kb, qb, yb = [None] * CT, [None] * CT, [None] * CT
    p2tags = ["pa", "po0", "po1", "po2", "po3"]
    p2cnt = [0]

    def p2psum():
        tag = p2tags[p2cnt[0] % len(p2tags)]
        p2cnt[0] += 1
        return qpool.tile([P, CHW], F32, tag=tag,
                          bufs=(3 if tag == "pa" else 1), name="p2")

    for ch in range(HW // CHW):
        sl = slice(ch * CHW, (ch + 1) * CHW)
        for ot in range(CT):
            pk = p2psum()
            for t in range(PT):
                nc.tensor.matmul(pk[:], ws8["wkT"][t][:, :, ot * P:(ot + 1) * P],
                                 x8[t][:, :, sl], start=(t == 0),
                                 stop=(t == PT - 1), perf_mode=DR)
            if ch == 0:
                kb[ot] = bias_ct("wkT", ot, F32, 3, "kb")
            nc.vector.tensor_scalar(k8[ot // 2][:, ot % 2, sl], pk[:],
                                    kb[ot][:, 0:1], None, OP.add)
        for nt in range(CT):
            jt = ch * CT + nt
            pv = p2psum()
            for t in range(PT):
                nc.tensor.matmul(pv[:], x8[t][:, :, ch * CHW + nt * P:
                                              ch * CHW + (nt + 1) * P],
                                 ws8["wvT"][t][:], start=(t == 0),
                                 stop=(t == PT - 1), perf_mode=DR)
            if ch == 0:
                bias_ct("wvT", nt, FP8, 4, "vbt")
            elif ch == 1:
                yb[nt] = yb_ct(nt)
            nc.scalar.copy(vT8[jt // 2][:, jt % 2, :], pv[:])
        if ch * CHW < QN:
            for ot in range(CT):
                pq = p2psum()
                for t in range(PT):
                    nc.tensor.matmul(pq[:],
                                     ws8["wqT"][t][:, :, ot * P:(ot + 1) * P],
                                     x8[t][:, :, sl], start=(t == 0),
                                     stop=(t == PT - 1), perf_mode=DR)
                if ch == 0:
                    qb[ot] = bias_ct("wqT", ot, F32, 2, "qb")
                nc.vector.tensor_scalar(q8[ot // 2][:, ot % 2, sl], pq[:],
                                        qb[ot][:, 0:1], None, OP.add)

    # x + yb = (x8 + d8) + yb, precomputed off the critical path for the y
    # epilogue
    xyb = [[None] * CT for _ in range(2)]
    for ih in range(2):
        for ot in range(CT):
            sl = slice(ih * CHW, (ih + 1) * CHW)
            T, e = ot // 2, ot % 2
            xt = ppool.tile([P, CHW], F32, tag=f"xyb{ih}{ot}",
                            name=f"xyb{ih}{ot}")
            nc.vector.tensor_scalar(xt[:], d8_sb[T][:, e, sl],
                                    yb[ot][:, 0:1], None, OP.add)
            nc.vector.tensor_tensor(xt[:], xt[:], x8[T][:, e, sl], op=OP.add)
            xyb[ih][ot] = xt

    # ---- phase 3: attention, per query half ----
    def mk_pr():
        return qpool.tile([1, CHW], F32, tag="pr", bufs=1, name="pr")

    def mk_po():
        return [qpool.tile([P, CHW], F32, tag=f"po{t}", name=f"po{t}", bufs=1)
                for t in range(CT)]

    def sc_exp(ih, j):
        isl = slice(ih * CHW, (ih + 1) * CHW)
        ps_ = qpool.tile([P, CHW], F32, tag="pa", name="ps")
        for t in range(PT):
            nc.tensor.matmul(ps_[:], k8[t][:, :, j * P:(j + 1) * P],
                             q8[t][:, :, isl], start=(t == 0),
                             stop=(t == PT - 1), perf_mode=DR)
        pT = spool.tile([P, 2, CHW], FP8, tag="pT", bufs=12, name="pT") \
            if j % 2 == 0 else None
        return ps_, pT

    def exp_into(pair, par, ps_):
        nc.scalar.activation(pair[:, par, :], ps_[:], AF.Exp, scale=SCALE,
                             bias=ebias[:, 0:1])

    def av_only(po, jj, pair):
        for t in range(CT):
            nc.tensor.matmul(po[t][:], vT8[jj][:, :, t * P:(t + 1) * P],
                             pair[:], start=(jj == 0), stop=(jj == JJ - 1),
                             perf_mode=DR)

    def rowsum_only(pr, jj, pair):
        nc.tensor.matmul(pr[:], ones8[:, :, 0:1], pair[:],
                         start=(jj == 0), stop=(jj == JJ - 1), perf_mode=DR)

    def tail_and_y(pr, po, ih):
        # 1/r chain first: pr closed early (rowsum bursts), so DVE computes
        # rinv/rbb and the broadcast matmul lands BEFORE the last AV
        # finishes; the softmax normalization folds into the fp8 'has'
        # tiles (has = po * (1/r)), so the epilogue is a single add
        rinv = smpool.tile([1, CHW], F32, tag="rinv", bufs=2)
        nc.vector.reciprocal_approx_fast(rinv[:], pr[:])
        rbb = smpool.tile([1, CHW], BF16, tag="rbb", bufs=2)
        nc.vector.tensor_copy(rbb[:], rinv[:])
        rb = spool.tile([P, CHW], F32, tag="rb", bufs=2)
        prb = qpool.tile([P, CHW], F32, tag="pa")
        nc.tensor.matmul(prb[:], ones_row[:], rbb[:], start=True, stop=True)
        # rb copy on ScalarE (Copy is table-neutral): frees DVE to start the
        # has multiplies the moment rb lands
        nc.scalar.copy(rb[:], prb[:])
        has = []
        for t in range(PT):
            ha = spool.tile([P, 2, CHW], FP8, tag=f"hx{t}", bufs=2)
            nc.vector.tensor_tensor(ha[:, 0, :], po[2 * t][:], rb[:],
                                    op=OP.mult)
            nc.vector.tensor_tensor(ha[:, 1, :], po[2 * t + 1][:], rb[:],
                                    op=OP.mult)
            has.append(ha)
        for ot in range(CT):
            # reuse the freed po slot: the pa slots stay available for the
            # next half's score pipeline even while the 1/r chain lags
            py = qpool.tile([P, CHW], F32, tag=f"po{ot}", name="py", bufs=1)
            for t in range(PT):
                nc.tensor.matmul(py[:], wp8[t][:, :, ot * P:(ot + 1) * P],
                                 has[t][:], start=(t == 0),
                                 stop=(t == PT - 1), perf_mode=DR)
            # column-split epilogue: each half fires its y DMA as soon as
            # its DVE add is done, hiding the per-transfer DMA latency
            yt = spool.tile([P, CHW], F32, tag="yt", bufs=3)
            for hc in range(2):
                cs = slice(hc * (CHW // 2), (hc + 1) * (CHW // 2))
                nc.vector.tensor_tensor(yt[:, cs], py[:, cs],
                                        xyb[ih][ot][:, cs], op=OP.add)
                # y spread over all 3 DMA queues (ScalarE is idle and its
                # ring long drained by the tails)
                dma3[(2 * ot + hc) % 3].dma_start(
                    y_d[ot, :, ih * CHW + hc * (CHW // 2):
                        ih * CHW + (hc + 1) * (CHW // 2)], yt[:, cs])

    # scores/exp run LA jj-pairs ahead of the AV that consumes the pair:
    # by the time the PE reaches each consumer, the exp's (late-posting)
    # completion semaphore is stale and the PE never waits on ScalarE. The
    # next half's first KPRE pair groups are emitted into the drain/tail
    # window so the PE never idles across halves.
    LA, KPRE = 2, 3
    pr0 = mk_pr()
    po0 = mk_po()
    pr1 = mk_pr()
    pairs0, pre = {}, {}
    for it in range(JJ + KPRE):
        if it < JJ:
            ps0, pair = sc_exp(0, 2 * it)
            exp_into(pair, 0, ps0)
            ps1, _ = sc_exp(0, 2 * it + 1)
            exp_into(pair, 1, ps1)
            pairs0[it] = pair
        elif it - JJ < KPRE:
            jjp = it - JJ
            ps0, pair = sc_exp(1, 2 * jjp)
            exp_into(pair, 0, ps0)
            ps1, _ = sc_exp(1, 2 * jjp + 1)
            exp_into(pair, 1, ps1)
            pre[jjp] = pair
        # rowsums in bursts of 8 pairs: the M=1 matmul costs PE reconfig on
        # each entry/exit, so amortize it; the last burst lands before the
        # last AV so the 1/r chain overlaps it
        if it in (JJ // 2, JJ):
            for jp in range(it - JJ // 2, it):
                rowsum_only(pr0, jp, pairs0[jp])
        ja = it - LA
        if 0 <= ja < JJ:
            av_only(po0, ja, pairs0[ja])
    tail_and_y(pr0, po0, 0)
    po1 = mk_po()
    for it in range(JJ):
        js = it + KPRE
        if js < JJ:
            ps0, pair = sc_exp(1, 2 * js)
            exp_into(pair, 0, ps0)
            ps1, _ = sc_exp(1, 2 * js + 1)
            exp_into(pair, 1, ps1)
            pre[js] = pair
        if it in (JJ // 2 - 1, JJ - 1):
            for jp in range(it - JJ // 2 + 1, it + 1):
                rowsum_only(pr1, jp, pre[jp])
        av_only(po1, it, pre[it])
    tail_and_y(pr1, po1, 1)


def build_module():
    nc = bacc.Bacc("TRN2", target_bir_lowering=False, debug=False,
                   num_devices=NCORES)
    d = {
        "x8": nc.dram_tensor("x8", [PT, 2, P, 2, HW // 2], FP8,
                             kind="ExternalInput").ap(),
        "xb": nc.dram_tensor("xb", [PT, 2, P, 2, HW // 2], FP8,
                             kind="ExternalInput").ap(),
        "wqT": nc.dram_tensor("wqT", [CT, P, C], BF16, kind="ExternalInput").ap(),
        "wkT": nc.dram_tensor("wkT", [CT, P, C], BF16, kind="ExternalInput").ap(),
        "wvT": nc.dram_tensor("wvT", [CT, P, C], BF16, kind="ExternalInput").ap(),
        "wp8": nc.dram_tensor("wp8", [PT, P, 2, C], FP8,
                              kind="ExternalInput").ap(),
        "wr8": nc.dram_tensor("wr8", [3, PT, P, 2, C], FP8,
                              kind="ExternalInput").ap(),
        "chv": nc.dram_tensor("chv", [P, 6 * CT], F32, kind="ExternalInput").ap(),
        "ind8": nc.dram_tensor("ind8", [P, 2, 16], FP8,
                               kind="ExternalInput").ap(),
        "indTA": nc.dram_tensor("indTA", [16, P], F32,
                                kind="ExternalInput").ap(),
        "indTB": nc.dram_tensor("indTB", [16, P], F32,
                                kind="ExternalInput").ap(),
        "y": nc.dram_tensor("y", [CT, P, QN], F32, kind="ExternalOutput").ap(),
    }
    with tile.TileContext(nc) as tc, ExitStack() as ctx:
        _build_body(nc, tc, ctx, d)
    nc.compile()
    return nc


_CACHE = {}


def _get_nc():
    if "nc" not in _CACHE:
        _CACHE["nc"] = build_module()
    return _CACHE["nc"]


def _shared_inputs(gamma, beta, wq, bq, wk, bk, wv, bv, wp, bp):
    import ml_dtypes

    def wT(w):
        wt = np.ascontiguousarray(np.asarray(w, np.float32).T)
        return wt.reshape(CT, P, C).astype(ml_dtypes.bfloat16)

    def w8(w):
        wt = np.asarray(w, np.float32).T.reshape(PT, 2, P, C)
        return np.ascontiguousarray(wt.transpose(0, 2, 1, 3)).astype(
            ml_dtypes.float8_e4m3)

    wp8 = w8(wp)
    wr8 = np.stack([w8(wk), w8(wv), w8(wq)])

    # group indicator for the PE stats matmuls: partition p / parity e of a
    # pair-tile belongs to group row 8*e + p//16
    ind8 = np.zeros((P, 2, 16), np.float32)
    for p in range(P):
        for e in range(2):
            ind8[p, e, 8 * e + p // 16] = 1.0
    # broadcast-back indicators (per parity): group row -> partition
    indTA = np.zeros((16, P), np.float32)
    indTB = np.zeros((16, P), np.float32)
    for p in range(P):
        indTA[p // 16, p] = 1.0
        indTB[8 + p // 16, p] = 1.0
    chv = np.stack([np.asarray(a, np.float32)
                    for a in (gamma, beta, bq, bk, bv, bp)],
                   axis=1).reshape(CT, P, 6).transpose(1, 0, 2).reshape(P, 24)
    return {
        "wqT": wT(wq), "wkT": wT(wk), "wvT": wT(wv), "wp8": wp8,
        "wr8": wr8,
        "chv": np.ascontiguousarray(chv),
        "ind8": ind8.astype(ml_dtypes.float8_e4m3),
        "indTA": indTA, "indTB": indTB,
    }


def make_in_maps(x, gamma, beta, wq, bq, wk, bk, wv, bv, wp, bp):
    import ml_dtypes

    shared = _shared_inputs(gamma, beta, wq, bq, wk, bk, wv, bv, wp, bp)
    xf = np.asarray(x, np.float32).reshape(B, C, HW)
    in_maps = []
    for core in range(NCORES):
        b, qc = divmod(core, NCORES // B)
        xr = np.roll(xf[b], -qc * QN, axis=1)          # [C, HW]
        # fp8 channel-pair layout [T, half, p, e, m]: c = 256T+128e+p
        xp = np.ascontiguousarray(
            xr.reshape(PT, 2, P, 2, HW // 2).transpose(0, 3, 2, 1, 4))
        x8 = xp.astype(ml_dtypes.float8_e4m3)
        d8 = (xp - x8.astype(np.float32)).astype(ml_dtypes.float8_e4m3)
        m = dict(shared)
        m["x8"] = x8
        m["xb"] = d8
        in_maps.append(m)
    return in_maps


def assemble_output(results):
    out = np.empty((B, C, HW), np.float32)
    for core in range(NCORES):
        b, qc = divmod(core, NCORES // B)
        y = np.asarray(results[core]["y"]).reshape(C, QN)
        out[b, :, qc * QN:(qc + 1) * QN] = y
    return out.reshape(B, C, H, W)


def kernel(x, gamma, beta, wq, bq, wk, bk, wv, bv, wp, bp):
    nc = _get_nc()
    in_maps = make_in_maps(x, gamma, beta, wq, bq, wk, bk, wv, bv, wp, bp)
    res = run_bass_kernel_spmd(nc, in_maps, list(range(NCORES)))
    return assemble_output(res.results)


# revision 32
# speedup vs baseline: 1.2366x; 1.1874x over previous
"""Trainium2 Bass kernel for nn_AttentionBlock (GroupNorm + single-head spatial
self-attention + residual) on 8 NeuronCores.

Sharding: data-parallel over batch (2) x sequence-parallel over the query
dimension (4 chunks of 1024 of the 4096 spatial tokens). Each core gets the
full image of its batch element, ROTATED so its query chunk sits at token 0
(GroupNorm stats, key/value sets and softmax sums are permutation-invariant
over tokens, so rotation lets all 8 cores run the identical SPMD program).

v2: every large matmul runs in fp8e4m3 with perf_mode=DoubleRow (two K=128
contraction tiles fused per instruction, ~1.44x PE throughput at FD>=512).
All pairs are [128, 2, N] tiles pairing adjacent 128-blocks of the
contraction dim. Measured end-to-end rel err ~9e-3 vs the 2e-2 gate.

  x ships as fp8 channel-pairs (2MB) and is the matmul operand for q/k/v;
  a second bf16 copy of x arrives later (DMA idle mid-kernel) only for the
  residual. GroupNorm stats come from the fp8 x via DVE free-dim reduces
  (sums) + ScalarE Square activations with accum_out (sumsq) streamed
  behind the DMAs; group-combine via tiny PE indicator matmuls as before.
  The GroupNorm scale a[c] folds into fp8 copies of wq/wk/wv (bf16 raw
  weights ship for the tiny bias contracts); the shift b[c] becomes
  per-output-channel constants (rows of w.b) as in v1.

  scores: sT[j,i] = sum_c k[c,j] q[c,i] as 2 DoubleRow matmuls (c-pairs);
  exp on ScalarE with a constant logit shift of -2.5 (softmax is shift
  invariant; the shift keeps exp under fp8 max 240; logit max is ~7.2)
  writing fp8 straight into j-pair tiles. AV + rowsum consume the pairs
  with DoubleRow (vT j-pairs / fp8 ones). Softmax normalization is
  deferred past the wp projection: po -> fp8 'has' pairs scaled by 1/16
  (keeps po under fp8 range; the x16 folds into the 1/r row broadcast,
  whose ones-row carries value 16). Final y = py*(16/r) + (x + yb).
"""

import sys
from contextlib import ExitStack

if "/opt/trn_rl_repo" not in sys.path:
    sys.path.insert(0, "/opt/trn_rl_repo")

import numpy as np

import concourse.bass as bass  # noqa: F401  (import keeps bass registered)
import concourse.tile as tile
from concourse import bacc, mybir
from concourse.alu_op_type import AluOpType
from concourse.bass_utils import run_bass_kernel_spmd

F32 = mybir.dt.float32
BF16 = mybir.dt.bfloat16
FP8 = mybir.dt.float8e4
AF = mybir.ActivationFunctionType
OP = AluOpType
DR = mybir.MatmulPerfMode.DoubleRow
AXX = mybir.AxisListType.X

B, C, H, W = 2, 512, 64, 64
HW = H * W          # 4096 spatial tokens
P = 128             # partitions
CT = C // P         # 4 channel tiles
PT = CT // 2        # 2 channel pair-tiles
NCORES = 8
QN = HW // 4        # 1024 queries per core
CHW = 512           # token chunk width
JT = HW // P        # 32 key tiles
JJ = JT // 2        # 16 key pair-tiles
EPS = 1e-6
SCALE = float(C) ** -0.5
SH = 2.5            # constant logit shift (softmax-invariant; fp8 range)


def _build_body(nc, tc, ctx, d):
    x8_d = d["x8"]
    xb_d = d["xb"]
    y_d = d["y"]

    cpool = ctx.enter_context(tc.tile_pool(name="const", bufs=1))
    ppool = ctx.enter_context(tc.tile_pool(name="persist", bufs=1))
    spool = ctx.enter_context(tc.tile_pool(name="stream", bufs=2))
    smpool = ctx.enter_context(tc.tile_pool(name="small", bufs=1))
    qpool = ctx.enter_context(tc.tile_pool(name="psum", bufs=3, space="PSUM"))

    # DMA queue assignment: ScalarE's instruction queue must stay clear for
    # the phase-1 Square stream (a DMA descriptor-gen blocked on ring space
    # would stall everything behind it), so ScalarE only issues a share of
    # the x8 transfers (needed earliest); weights, consts and the low-
    # priority bf16 x ride on gpsimd/sync.
    bulk_engines = [nc.gpsimd, nc.sync]

    # small constants first (tiny; must not sit behind bulk transfers)
    ind8 = cpool.tile([P, 2, 16], FP8, tag="ind8")
    nc.gpsimd.dma_start(ind8[:], d["ind8"][:])
    indTA = cpool.tile([16, P], F32, tag="indTA")
    nc.sync.dma_start(indTA[:], d["indTA"][:])
    indTB = cpool.tile([16, P], F32, tag="indTB")
    nc.sync.dma_start(indTB[:], d["indTB"][:])
    chv24 = cpool.tile([P, 6 * CT], F32, tag="chv24")
    nc.gpsimd.dma_start(chv24[:], d["chv"][:])

    # ---- phase 1: x arrives fp8 in channel-pair layout (half-tile
    # transfers with 2KB contiguous rows, earliest tokens first) and is the
    # matmul operand for the whole kernel
    x8 = [ppool.tile([P, 2, HW], FP8, tag=f"x8{t}", name=f"x8{t}")
          for t in range(PT)]
    dma3 = [nc.gpsimd, nc.scalar, nc.sync]
    HN = HW // 2
    for hf in range(2):
        for t in range(PT):
            eng = dma3[(hf * PT + t) % 3]
            eng.dma_start(x8[t][:, :, hf * HN:(hf + 1) * HN], x8_d[t, hf])
    # chv columns per tile t at 6*t+j: gamma, beta, bq, bk, bv, bp
    chv3 = chv24.rearrange("p (t six) -> p t six", six=6)

    def chvcol(t, c):
        return chv24[:, 6 * t + c:6 * t + c + 1]
    # bulk weights, in consumption order, on the gpsimd/sync queues
    wts = {}
    for wi, name in enumerate(("wkT", "wvT", "wqT")):
        wts[name] = []
        for t in range(CT):
            w = cpool.tile([P, C], BF16, tag=f"{name}{t}", name=f"{name}{t}")
            bulk_engines[(wi * CT + t) % 2].dma_start(w[:], d[name][t])
            wts[name].append(w)
    wp8 = []
    for t in range(PT):
        w = cpool.tile([P, 2, C], FP8, tag=f"wp8{t}", name=f"wp8{t}")
        bulk_engines[t % 2].dma_start(w[:], d["wp8"][t])
        wp8.append(w)
    # fp8 delta of x (x ~= x8 + d8, residual error below bf16 level) for
    # the residual path only -- needed from the xyb precompute on, so it
    # rides last on the gpsimd/sync queues and is half the bf16 bytes
    d8_sb = [ppool.tile([P, 2, HW], FP8, tag=f"d8{t}", name=f"d8{t}")
             for t in range(PT)]
    for t in range(PT):
        for hf in range(2):
            bulk_engines[(t * 2 + hf) % 2].dma_start(
                d8_sb[t][:, :, hf * HN:(hf + 1) * HN], xb_d[t, hf])

    ones8 = cpool.tile([P, 2, 16], FP8, tag="ones8")
    nc.vector.memset(ones8[:], 1.0)
    ones_row = smpool.tile([1, P], BF16, tag="onesr")
    nc.vector.memset(ones_row[:], 1.0)
    epst16 = smpool.tile([16, 2], F32, tag="eps")
    nc.vector.memset(epst16[:], EPS)
    ebias = smpool.tile([P, 1], F32, tag="ebias")
    nc.vector.memset(ebias[:], -SH)
    i16hw = smpool.tile([16, 1], F32, tag="i16hw")
    nc.vector.memset(i16hw[:], 1.0 / (16.0 * HW))
    dumt = smpool.tile([16, 1], F32, tag="dumt")

    # GroupNorm stats on the otherwise-idle PE: per 512-token chunk, a
    # DoubleRow matmul with a group-indicator lhsT accumulates per-group
    # token-position sums in PSUM; a second accumulator consumes x^2 chunks
    # produced by a square pass split across ScalarE (Square activation)
    # and DVE (tensor_tensor mult). Group g of pair-tile T sits at PSUM
    # row 8*parity + group-within-parity.
    psgS = [qpool.tile([16, CHW], F32, tag=f"po{T}", bufs=1, name=f"psgS{T}")
            for T in range(PT)]
    psgQ = [qpool.tile([16, CHW], F32, tag=f"po{2 + T}", bufs=1,
                       name=f"psgQ{T}") for T in range(PT)]
    sqjob = 0
    for qtr in range(4):
        for T in range(PT):
            for h in range(2):
                ch = 2 * qtr + h
                sl = slice(ch * CHW, (ch + 1) * CHW)
                nc.tensor.matmul(psgS[T][:], ind8[:], x8[T][:, :, sl],
                                 start=(ch == 0), stop=(ch == 7),
                                 perf_mode=DR)
        for T in range(PT):
            for h in range(2):
                ch = 2 * qtr + h
                sl = slice(ch * CHW, (ch + 1) * CHW)
                # square pass split ScalarE/DVE (GpSimd measured 2-3x
                # slower per op and its queue is busy with DMA gens)
                eng = sqjob % 2
                sqjob += 1
                xq = spool.tile([P, 2, CHW], FP8,
                                tag=f"xsq{eng}", bufs=3)
                if eng == 0:
                    nc.scalar.activation(xq[:], x8[T][:, :, sl], AF.Square)
                else:
                    nc.vector.tensor_tensor(xq[:], x8[T][:, :, sl],
                                            x8[T][:, :, sl], op=OP.mult)
                nc.tensor.matmul(psgQ[T][:], ind8[:], xq[:],
                                 start=(ch == 0), stop=(ch == 7),
                                 perf_mode=DR)
    # preload the Sqrt table right behind the last Square (the combine
    # chain below hides the 1.3us load)
    nc.scalar.activation(dumt[:], epst16[:, 0:1], AF.Sqrt)
    s2 = smpool.tile([16, PT, 2], F32, tag="s2")
    # S-reduces first: psgS closes with the last x8 chunk, well before the
    # square pass drains, so they run under the square tail
    for T in range(PT):
        nc.vector.tensor_reduce(s2[:, T, 0:1], psgS[T][:], AXX, OP.add)
    for T in range(PT):
        nc.vector.tensor_reduce(s2[:, T, 1:2], psgQ[T][:], AXX, OP.add)
    s2m = smpool.tile([16, PT, 2], F32, tag="s2m")   # [mu, E[x^2]] per T
    nc.vector.tensor_scalar(s2m[:], s2[:], i16hw[:, 0:1], None, OP.mult)
    msq = smpool.tile([16, PT], F32, tag="msq")
    nc.vector.tensor_tensor(msq[:], s2m[:, :, 0], s2m[:, :, 0], op=OP.mult)
    varg = smpool.tile([16, PT], F32, tag="varg")
    nc.vector.tensor_tensor(varg[:], s2m[:, :, 1], msq[:], op=OP.subtract)
    stdg = smpool.tile([16, PT], F32, tag="stdg")
    nc.scalar.activation(stdg[:], varg[:], AF.Sqrt, bias=epst16[:, 0:1])
    # preload the Exp table right after the last Sqrt (copies in between
    # are table-neutral), so phase 3's first exp starts without a reload
    nc.scalar.activation(dumt[:], stdg[:, 0:1], AF.Exp)
    # interleave (mu_T, rstd_T) columns and broadcast groups->channels with
    # two [K=16, M=128, N=4] indicator matmuls (one per channel parity)
    mr = smpool.tile([16, PT, 2], F32, tag="mr")
    nc.vector.tensor_copy(mr[:, :, 0], s2m[:, :, 0])
    nc.vector.reciprocal(mr[:, :, 1], stdg[:])
    cbA = qpool.tile([P, 2 * PT], F32, tag="pa")
    nc.tensor.matmul(cbA[:], indTA[:], mr[:], start=True, stop=True)
    cbB = qpool.tile([P, 2 * PT], F32, tag="pa")
    nc.tensor.matmul(cbB[:], indTB[:], mr[:], start=True, stop=True)
    # cb4[p, t, {mu,rstd}] with t = 2T + parity
    cb4 = smpool.tile([P, CT, 2], F32, tag="cb4")
    cb4v = cb4.rearrange("p (T e) two -> p T e two", e=2)
    nc.vector.tensor_copy(cb4v[:, :, 0, :], cbA[:])
    nc.vector.tensor_copy(cb4v[:, :, 1, :], cbB[:])

    # per-channel Scale a / Bias b, vectorized across all 4 tiles via
    # strided views (one wide DVE op instead of one per tile)
    ab = ppool.tile([P, 2, CT], F32, tag="ab")   # [:,0,t]=a_t  [:,1,t]=b_t
    nc.vector.tensor_tensor(ab[:, 0, :], cb4[:, :, 1], chv3[:, :, 0],
                            op=OP.mult)
    tmpb = smpool.tile([P, CT], F32, tag="tmpb", bufs=1)
    nc.vector.tensor_tensor(tmpb[:], cb4[:, :, 0], ab[:, 0, :], op=OP.mult)
    nc.vector.tensor_tensor(ab[:, 1, :], chv3[:, :, 1], tmpb[:],
                            op=OP.subtract)
    bvec_all = ppool.tile([P, CT, 2], BF16, tag="bva")
    nc.vector.tensor_copy(bvec_all[:, :, 0], ab[:, 1, :])
    nc.vector.tensor_copy(bvec_all[:, :, 1], ab[:, 1, :])
    sbts = [ab[:, 0, t:t + 1] for t in range(CT)]
    bvec = [bvec_all[:, t, :] for t in range(CT)]

    # GroupNorm scale folded into fp8 PAIR COPIES of the projection weights
    # (the raw bf16 tiles stay live for the bias contracts interleaved into
    # phase 2 below); split DVE/ScalarE (activation-Copy is table-neutral),
    # wk first so the first k-projection chunk is gated as early as possible
    ws8 = {}
    for name in ("wkT", "wvT", "wqT"):
        ws8[name] = []
        for t in range(PT):
            w = cpool.tile([P, 2, C], FP8, tag=f"s{name}{t}",
                           name=f"s{name}{t}")
            nc.vector.tensor_scalar_mul(w[:, 0, :], wts[name][2 * t][:],
                                        sbts[2 * t])
            nc.scalar.activation(w[:, 1, :], wts[name][2 * t + 1][:],
                                 AF.Copy, scale=sbts[2 * t + 1])
            ws8[name].append(w)

    # ---- bias-term constants from RAW weights (tiny N=2 matmuls);
    # emitted one output-tile group at a time, interleaved into phase 2's
    # first chunks so they never head-of-line block the projections ----
    #   qb[o] = sum_c wq[o,c] b[c] + bq    (added at the q PSUM->SBUF move)
    #   kb[o] = likewise with bk
    #   vb8[c] = sum_cin wv[c,cin] b[cin] + bv   (rides softmax into yb)
    #   yb[o] = sum_c wp[o,c] vb8[c] + bp        (y epilogue constant)
    vb8 = [ppool.tile([P, 2, 16], FP8, tag=f"vb8{t}", name=f"vb8{t}")
           for t in range(PT)]

    def bias_ct(wname, ot, outdt, addcol, tagp):
        pb = qpool.tile([P, 2], F32, tag="pa", name="pb")
        for t in range(CT):
            nc.tensor.matmul(pb[:], wts[wname][t][:, ot * P:(ot + 1) * P],
                             bvec[t][:, 0:2], start=(t == 0),
                             stop=(t == CT - 1))
        if outdt == F32:
            ob = ppool.tile([P, 1], F32, tag=f"{tagp}{ot}", name=f"{tagp}{ot}")
            nc.vector.tensor_scalar(ob[:], pb[:, 0:1], chvcol(ot, addcol),
                                    None, OP.add)
            return ob
        # fp8 pair column for the yb contract
        tf = smpool.tile([P, 1], F32, tag="tf", bufs=2)
        nc.vector.tensor_scalar(tf[:], pb[:, 0:1], chvcol(ot, addcol),
                                None, OP.add)
        nc.vector.tensor_copy(vb8[ot // 2][:, ot % 2, 0:1], tf[:])
        nc.vector.tensor_copy(vb8[ot // 2][:, ot % 2, 1:2], tf[:])
        return None

    def yb_ct(ot):
        pb = qpool.tile([P, 2], F32, tag="pa", name="pb")
        for t in range(PT):
            nc.tensor.matmul(pb[:], wp8[t][:, :, ot * P:(ot + 1) * P],
                             vb8[t][:, :, 0:2], start=(t == 0),
                             stop=(t == PT - 1), perf_mode=DR)
        ob = ppool.tile([P, 1], F32, tag=f"yb{ot}", name=f"yb{ot}")
        nc.vector.tensor_scalar(ob[:], pb[:, 0:1], chvcol(ot, 5),
                                None, OP.add)
        return ob

    # ---- persistent attention operands (all fp8 pairs) ----
    k8 = [ppool.tile([P, 2, HW], FP8, tag=f"k8{t}", name=f"k8{t}")
          for t in range(PT)]
    q8 = [ppool.tile([P, 2, QN], FP8, tag=f"q8{t}", name=f"q8{t}")
          for t in range(PT)]
    vT8 = [ppool.tile([P, 2, C], FP8, tag=f"vT8{j}", name=f"vT8{j}")
           for j in range(JJ)]

    # ---- phase 2: q/k/v projections straight from resident fp8 x; the
    # bias contracts ride along inside chunks 0-1 ----
    kb, qb, yb = [None] * CT, [None] * CT, [None] * CT
    p2tags = ["pa", "po0", "po1", "po2", "po3"]
    p2cnt = [0]

    def p2psum():
        tag = p2tags[p2cnt[0] % len(p2tags)]
        p2cnt[0] += 1
        return qpool.tile([P, CHW], F32, tag=tag,
                          bufs=(3 if tag == "pa" else 1), name="p2")

    for ch in range(HW // CHW):
        sl = slice(ch * CHW, (ch + 1) * CHW)
        for ot in range(CT):
            pk = p2psum()
            for t in range(PT):
                nc.tensor.matmul(pk[:], ws8["wkT"][t][:, :, ot * P:(ot + 1) * P],
                                 x8[t][:, :, sl], start=(t == 0),
                                 stop=(t == PT - 1), perf_mode=DR)
            if ch == 0:
                kb[ot] = bias_ct("wkT", ot, F32, 3, "kb")
            nc.vector.tensor_scalar(k8[ot // 2][:, ot % 2, sl], pk[:],
                                    kb[ot][:, 0:1], None, OP.add)
        for nt in range(CT):
            jt = ch * CT + nt
            pv = p2psum()
            for t in range(PT):
                nc.tensor.matmul(pv[:], x8[t][:, :, ch * CHW + nt * P:
                                              ch * CHW + (nt + 1) * P],
                                 ws8["wvT"][t][:], start=(t == 0),
                                 stop=(t == PT - 1), perf_mode=DR)
            if ch == 0:
                bias_ct("wvT", nt, FP8, 4, "vbt")
            elif ch == 1:
                yb[nt] = yb_ct(nt)
            nc.scalar.copy(vT8[jt // 2][:, jt % 2, :], pv[:])
        if ch * CHW < QN:
            for ot in range(CT):
                pq = p2psum()
                for t in range(PT):
                    nc.tensor.matmul(pq[:],
                                     ws8["wqT"][t][:, :, ot * P:(ot + 1) * P],
                                     x8[t][:, :, sl], start=(t == 0),
                                     stop=(t == PT - 1), perf_mode=DR)
                if ch == 0:
                    qb[ot] = bias_ct("wqT", ot, F32, 2, "qb")
                nc.vector.tensor_scalar(q8[ot // 2][:, ot % 2, sl], pq[:],
                                        qb[ot][:, 0:1], None, OP.add)

    # x + yb = (x8 + d8) + yb, precomputed off the critical path for the y
    # epilogue
    xyb = [[None] * CT for _ in range(2)]
    for ih in range(2):
        for ot in range(CT):
            sl = slice(ih * CHW, (ih + 1) * CHW)
            T, e = ot // 2, ot % 2
            xt = ppool.tile([P, CHW], F32, tag=f"xyb{ih}{ot}",
                            name=f"xyb{ih}{ot}")
            nc.vector.tensor_scalar(xt[:], d8_sb[T][:, e, sl],
                                    yb[ot][:, 0:1], None, OP.add)
            nc.vector.tensor_tensor(xt[:], xt[:], x8[T][:, e, sl], op=OP.add)
            xyb[ih][ot] = xt

    # ---- phase 3: attention, per query half ----
    def mk_pr():
        return qpool.tile([1, CHW], F32, tag="pr", bufs=1, name="pr")

    def mk_po():
        return [qpool.tile([P, CHW], F32, tag=f"po{t}", name=f"po{t}", bufs=1)
                for t in range(CT)]

    def sc_exp(ih, j):
        isl = slice(ih * CHW, (ih + 1) * CHW)
        ps_ = qpool.tile([P, CHW], F32, tag="pa", name="ps")
        for t in range(PT):
            nc.tensor.matmul(ps_[:], k8[t][:, :, j * P:(j + 1) * P],
                             q8[t][:, :, isl], start=(t == 0),
                             stop=(t == PT - 1), perf_mode=DR)
        pT = spool.tile([P, 2, CHW], FP8, tag="pT", bufs=12, name="pT") \
            if j % 2 == 0 else None
        return ps_, pT

    def exp_into(pair, par, ps_):
        nc.scalar.activation(pair[:, par, :], ps_[:], AF.Exp, scale=SCALE,
                             bias=ebias[:, 0:1])

    def av_only(po, jj, pair):
        for t in range(CT):
            nc.tensor.matmul(po[t][:], vT8[jj][:, :, t * P:(t + 1) * P],
                             pair[:], start=(jj == 0), stop=(jj == JJ - 1),
                             perf_mode=DR)

    def rowsum_only(pr, jj, pair):
        nc.tensor.matmul(pr[:], ones8[:, :, 0:1], pair[:],
                         start=(jj == 0), stop=(jj == JJ - 1), perf_mode=DR)

    def tail_and_y(pr, po, ih):
        # 1/r chain first: pr closed early (rowsum bursts), so DVE computes
        # rinv/rbb and the broadcast matmul lands BEFORE the last AV
        # finishes; the softmax normalization folds into the fp8 'has'
        # tiles (has = po * (1/r)), so the epilogue is a single add
        rinv = smpool.tile([1, CHW], F32, tag="rinv", bufs=2)
        nc.vector.reciprocal_approx_fast(rinv[:], pr[:])
        rbb = smpool.tile([1, CHW], BF16, tag="rbb", bufs=2)
        nc.vector.tensor_copy(rbb[:], rinv[:])
        rb = spool.tile([P, CHW], F32, tag="rb", bufs=2)
        prb = qpool.tile([P, CHW], F32, tag="pa")
        nc.tensor.matmul(prb[:], ones_row[:], rbb[:], start=True, stop=True)
        # rb copy on ScalarE (Copy is table-neutral): frees DVE to start the
        # has multiplies the moment rb lands
        nc.scalar.copy(rb[:], prb[:])
        has = []
        for t in range(PT):
            ha = spool.tile([P, 2, CHW], FP8, tag=f"hx{t}", bufs=2)
            nc.vector.tensor_tensor(ha[:, 0, :], po[2 * t][:], rb[:],
                                    op=OP.mult)
            nc.vector.tensor_tensor(ha[:, 1, :], po[2 * t + 1][:], rb[:],
                                    op=OP.mult)
            has.append(ha)
        for ot in range(CT):
            # reuse the freed po slot: the pa slots stay available for the
            # next half's score pipeline even while the 1/r chain lags
            py = qpool.tile([P, CHW], F32, tag=f"po{ot}", name="py", bufs=1)
            for t in range(PT):
                nc.tensor.matmul(py[:], wp8[t][:, :, ot * P:(ot + 1) * P],
                                 has[t][:], start=(t == 0),
                                 stop=(t == PT - 1), perf_mode=DR)
            # column-split epilogue: each half fires its y DMA as soon as
            # its DVE add is done, hiding the per-transfer DMA latency
            yt = spool.tile([P, CHW], F32, tag="yt", bufs=3)
            for hc in range(2):
                cs = slice(hc * (CHW // 2), (hc + 1) * (CHW // 2))
                nc.vector.tensor_tensor(yt[:, cs], py[:, cs],
                                        xyb[ih][ot][:, cs], op=OP.add)
                # y spread over all 3 DMA queues (ScalarE is idle and its
                # ring long drained by the tails)
                dma3[(2 * ot + hc) % 3].dma_start(
                    y_d[ot, :, ih * CHW + hc * (CHW // 2):
                        ih * CHW + (hc + 1) * (CHW // 2)], yt[:, cs])

    # scores/exp run LA jj-pairs ahead of the AV that consumes the pair:
    # by the time the PE reaches each consumer, the exp's (late-posting)
    # completion semaphore is stale and the PE never waits on ScalarE. The
    # next half's first KPRE pair groups are emitted into the drain/tail
    # window so the PE never idles across halves.
    LA, KPRE = 2, 3
    pr0 = mk_pr()
    po0 = mk_po()
    pr1 = mk_pr()
    pairs0, pre = {}, {}
    for it in range(JJ + KPRE):
        if it < JJ:
            ps0, pair = sc_exp(0, 2 * it)
            exp_into(pair, 0, ps0)
            ps1, _ = sc_exp(0, 2 * it + 1)
            exp_into(pair, 1, ps1)
            pairs0[it] = pair
        elif it - JJ < KPRE:
            jjp = it - JJ
            ps0, pair = sc_exp(1, 2 * jjp)
            exp_into(pair, 0, ps0)
            ps1, _ = sc_exp(1, 2 * jjp + 1)
            exp_into(pair, 1, ps1)
            pre[jjp] = pair
        # rowsums in bursts of 8 pairs: the M=1 matmul costs PE reconfig on
        # each entry/exit, so amortize it; the last burst lands before the
        # last AV so the 1/r chain overlaps it
        if it in (JJ // 2, JJ):
            for jp in range(it - JJ // 2, it):
                rowsum_only(pr0, jp, pairs0[jp])
        ja = it - LA
        if 0 <= ja < JJ:
            av_only(po0, ja, pairs0[ja])
    tail_and_y(pr0, po0, 0)
    po1 = mk_po()
    for it in range(JJ):
        js = it + KPRE
        if js < JJ:
            ps0, pair = sc_exp(1, 2 * js)
            exp_into(pair, 0, ps0)
            ps1, _ = sc_exp(1, 2 * js + 1)
            exp_into(pair, 1, ps1)
            pre[js] = pair
        if it in (JJ // 2 - 1, JJ - 1):
            for jp in range(it - JJ // 2 + 1, it + 1):
                rowsum_only(pr1, jp, pre[jp])
        av_only(po1, it, pre[it])
    tail_and_y(pr1, po1, 1)


def build_module():
    nc = bacc.Bacc("TRN2", target_bir_lowering=False, debug=False,
                   num_devices=NCORES)
    d = {
        "x8": nc.dram_tensor("x8", [PT, 2, P, 2, HW // 2], FP8,
                             kind="ExternalInput").ap(),
        "xb": nc.dram_tensor("xb", [PT, 2, P, 2, HW // 2], FP8,
                             kind="ExternalInput").ap(),
        "wqT": nc.dram_tensor("wqT", [CT, P, C], BF16, kind="ExternalInput").ap(),
        "wkT": nc.dram_tensor("wkT", [CT, P, C], BF16, kind="ExternalInput").ap(),
        "wvT": nc.dram_tensor("wvT", [CT, P, C], BF16, kind="ExternalInput").ap(),
        "wp8": nc.dram_tensor("wp8", [PT, P, 2, C], FP8,
                              kind="ExternalInput").ap(),
        "chv": nc.dram_tensor("chv", [P, 6 * CT], F32, kind="ExternalInput").ap(),
        "ind8": nc.dram_tensor("ind8", [P, 2, 16], FP8,
                               kind="ExternalInput").ap(),
        "indTA": nc.dram_tensor("indTA", [16, P], F32,
                                kind="ExternalInput").ap(),
        "indTB": nc.dram_tensor("indTB", [16, P], F32,
                                kind="ExternalInput").ap(),
        "y": nc.dram_tensor("y", [CT, P, QN], F32, kind="ExternalOutput").ap(),
    }
    with tile.TileContext(nc) as tc, ExitStack() as ctx:
        _build_body(nc, tc, ctx, d)
    nc.compile()
    return nc


_CACHE = {}


def _get_nc():
    if "nc" not in _CACHE:
        _CACHE["nc"] = build_module()
    return _CACHE["nc"]


def _shared_inputs(gamma, beta, wq, bq, wk, bk, wv, bv, wp, bp):
    import ml_dtypes

    def wT(w):
        wt = np.ascontiguousarray(np.asarray(w, np.float32).T)
        return wt.reshape(CT, P, C).astype(ml_dtypes.bfloat16)

    def w8(w):
        wt = np.asarray(w, np.float32).T.reshape(PT, 2, P, C)
        return np.ascontiguousarray(wt.transpose(0, 2, 1, 3)).astype(
            ml_dtypes.float8_e4m3)

    wp8 = w8(wp)

    # group indicator for the PE stats matmuls: partition p / parity e of a
    # pair-tile belongs to group row 8*e + p//16
    ind8 = np.zeros((P, 2, 16), np.float32)
    for p in range(P):
        for e in range(2):
            ind8[p, e, 8 * e + p // 16] = 1.0
    # broadcast-back indicators (per parity): group row -> partition
    indTA = np.zeros((16, P), np.float32)
    indTB = np.zeros((16, P), np.float32)
    for p in range(P):
        indTA[p // 16, p] = 1.0
        indTB[8 + p // 16, p] = 1.0
    chv = np.stack([np.asarray(a, np.float32)
                    for a in (gamma, beta, bq, bk, bv, bp)],
                   axis=1).reshape(CT, P, 6).transpose(1, 0, 2).reshape(P, 24)
    return {
        "wqT": wT(wq), "wkT": wT(wk), "wvT": wT(wv), "wp8": wp8,
        "chv": np.ascontiguousarray(chv),
        "ind8": ind8.astype(ml_dtypes.float8_e4m3),
        "indTA": indTA, "indTB": indTB,
    }


def make_in_maps(x, gamma, beta, wq, bq, wk, bk, wv, bv, wp, bp):
    import ml_dtypes

    shared = _shared_inputs(gamma, beta, wq, bq, wk, bk, wv, bv, wp, bp)
    xf = np.asarray(x, np.float32).reshape(B, C, HW)
    in_maps = []
    for core in range(NCORES):
        b, qc = divmod(core, NCORES // B)
        xr = np.roll(xf[b], -qc * QN, axis=1)          # [C, HW]
        # fp8 channel-pair layout [T, half, p, e, m]: c = 256T+128e+p
        xp = np.ascontiguousarray(
            xr.reshape(PT, 2, P, 2, HW // 2).transpose(0, 3, 2, 1, 4))
        x8 = xp.astype(ml_dtypes.float8_e4m3)
        d8 = (xp - x8.astype(np.float32)).astype(ml_dtypes.float8_e4m3)
        m = dict(shared)
        m["x8"] = x8
        m["xb"] = d8
        in_maps.append(m)
    return in_maps


def assemble_output(results):
    out = np.empty((B, C, HW), np.float32)
    for core in range(NCORES):
        b, qc = divmod(core, NCORES // B)
        y = np.asarray(results[core]["y"]).reshape(C, QN)
        out[b, :, qc * QN:(qc + 1) * QN] = y
    return out.reshape(B, C, H, W)


def kernel(x, gamma, beta, wq, bq, wk, bk, wv, bv, wp, bp):
    nc = _get_nc()
    in_maps = make_in_maps(x, gamma, beta, wq, bq, wk, bk, wv, bv, wp, bp)
    res = run_bass_kernel_spmd(nc, in_maps, list(range(NCORES)))
    return assemble_output(res.results)
